# revision 5
# baseline (speedup 1.0000x reference)
"""Trainium2 Bass kernel for nn_ConvBN2d (spiking CNN block).

Per-sample work (data-parallel over N=8 across 8 cores):
  - 20 timesteps of 3x3 conv (32->64ch, 64x64, BN-folded weights) + maxpool2x2
    + sequential spike recurrence, plus 1 ANN image (conv + BN + maxpool + relu).

Device strategy per core:
  - Host pre-shifts the input into 3 kx-shifted copies -> X3 [96, 4096]
    (zero-padded at the w boundaries), so the 3x3 conv becomes 3 PSUM-accumulated
    matmuls per 512-pixel chunk: K=96 (kx,cin), M=64 (cout), with the ky shift
    expressed as a +-64 offset into X3's free dim. float32r matmuls (1 cyc/row).
  - maxpool: pool-h on DVE directly from PSUM (stride-2 tensor_tensor max),
    pool-v on DVE from SBUF.
  - spike recurrence reformulated as first-threshold-crossing detection:
      c_t = cumsum(pooled_t);  f_t = (c_t >= thr_t);  g_t = max(g_{t-1}, f_t);
      s_t = g_t - g_{t-1};  count = g_19
    with thr_t[p] = 1 - (t+1)*bN[p] folding the conv bias out of the scan.
  - pooled maps packed [64,1024] -> [128,512] via 2 partition-offset DMAs so the
    scan runs on all 128 partitions.
"""

import sys
import types

if "/opt/trn_rl_repo" not in sys.path:
    sys.path.insert(0, "/opt/trn_rl_repo")

import numpy as np

import concourse.bacc as bacc
import concourse.tile as tile
from concourse import mybir
from concourse.bass_utils import run_bass_kernel_spmd

# Problem constants (hardcoded per contract)
N, T, CIN, COUT, H, W = 8, 20, 32, 64, 64, 64
K, PAD, POOL = 3, 1, 2
EPS = 1e-5
OH, OW = H // POOL, W // POOL  # 32, 32
HW = H * W                     # 4096
NPOOL = COUT * OH * OW         # 65536 pooled elems -> [64, 1024] -> packed [128, 512]

F32 = mybir.dt.float32
F32R = mybir.dt.float32r
BF16 = mybir.dt.bfloat16

MM_DT = F32R          # matmul compute dtype (bitcast view of fp32 bits)
CHUNK = 512           # conv free-dim chunk (one PSUM bank)
GROUP = 4             # chunks per PSUM tile (4 banks)
N_CHUNK = HW // CHUNK # 8
N_GROUP = N_CHUNK // GROUP  # 2

_COMPILED = None


def _mm(ap):
    return ap


def _conv_image(nc, tc, pools, wt_sel, x3, ph):
    """Emit matmuls + pool-h for one image. x3: [96, 4096] SBUF tile.
    wt_sel: [96, 3, 64] weight AP (ky-indexed). ph: [64, 2048] pool-h output."""
    psum_pool = pools["psum"]
    for g in range(N_GROUP):
        ps = psum_pool.tile([COUT, GROUP * CHUNK], F32)
        for ci in range(GROUP):
            cidx = g * GROUP + ci
            base = cidx * CHUNK
            # ky order: full-width matmul (ky=1) first so start=True covers
            # the whole chunk; edge-clipped ky=0/2 accumulate after.
            for j, ky in enumerate((1, 0, 2)):
                off = base + (ky - 1) * W
                lo = max(off, 0)
                hi = min(off + CHUNK, HW)
                n = hi - lo
                po = ci * CHUNK + (lo - off)
                nc.tensor.matmul(
                    ps[:, po:po + n],
                    lhsT=_mm(wt_sel[:, ky, :]),
                    rhs=_mm(x3[:, lo:hi]),
                    start=(j == 0),
                    stop=(j == 2),
                )
        # pool-h over this 4-bank group: [64, 2048] -> [64, 1024].
        # Only one tensor_tensor operand may come from PSUM, so ScalarE
        # evacuates the even-pixel stride to SBUF first.
        psv = ps.rearrange("p (f two) -> p f two", two=2)
        tmp = pools["phtmp"].tile([COUT, GROUP * CHUNK // 2], F32, tag="phtmp")
        nc.scalar.copy(out=tmp[:], in_=psv[:, :, 0])
        nc.vector.tensor_tensor(
            ph[:, g * (GROUP * CHUNK // 2):(g + 1) * (GROUP * CHUNK // 2)],
            tmp[:], psv[:, :, 1], mybir.AluOpType.max,
        )


def _pool_v(nc, ph, pv):
    """pool-v: ph [64, 2048] (h*32+u layout) -> pv [64, 1024] (v*32+u)."""
    phv = ph.rearrange("p (h par u) -> p h par u", par=2, u=OW)
    nc.vector.tensor_tensor(
        pv[:], phv[:, :, 0, :], phv[:, :, 1, :], mybir.AluOpType.max,
    )


def build_nc():
    nc = bacc.Bacc("TRN2", target_bir_lowering=False)

    # Per-core DRAM I/O (x3/wt typed float32r end-to-end: walrus requires
    # fp32r matmul operands to be produced as fp32r)
    x3_d = nc.dram_tensor("x3", [T, 96, HW], MM_DT, kind="ExternalInput")
    xsc3_d = nc.dram_tensor("xsc3", [96, HW], MM_DT, kind="ExternalInput")
    wt_d = nc.dram_tensor("wt", [2, 96, 3, COUT], MM_DT, kind="ExternalInput")
    thr_d = nc.dram_tensor("thr", [128, T], F32, kind="ExternalInput")
    aff_d = nc.dram_tensor("aff", [2, COUT], F32, kind="ExternalInput")

    spike_d = nc.dram_tensor("spike", [T, 128, 512], BF16, kind="ExternalOutput")
    count_d = nc.dram_tensor("count", [128, 512], BF16, kind="ExternalOutput")
    ann_d = nc.dram_tensor("ann", [COUT, OH * OW], F32, kind="ExternalOutput")

    with tile.TileContext(nc) as tc:
        from contextlib import ExitStack
        with ExitStack() as ctx:
            singles = ctx.enter_context(tc.tile_pool(name="singles", bufs=1))
            x3p = ctx.enter_context(tc.tile_pool(name="x3p", bufs=2))
            psum = ctx.enter_context(tc.tile_pool(name="psum", bufs=2, space="PSUM"))
            php = ctx.enter_context(tc.tile_pool(name="php", bufs=2))
            pvp = ctx.enter_context(tc.tile_pool(name="pvp", bufs=2))
            pkp = ctx.enter_context(tc.tile_pool(name="pkp", bufs=2))
            fp_ = ctx.enter_context(tc.tile_pool(name="fp", bufs=2))
            spp = ctx.enter_context(tc.tile_pool(name="spp", bufs=3))
            phtmp = ctx.enter_context(tc.tile_pool(name="phtmp", bufs=2))
            pools = {"psum": psum, "phtmp": phtmp}

            # --- persistent tiles ---
            wt_sb = singles.tile([96, 2, 3, COUT], MM_DT)
            nc.sync.dma_start(out=wt_sb[:], in_=wt_d.rearrange("s p ky co -> p s ky co"))
            thr_sb = singles.tile([128, T], F32)
            nc.sync.dma_start(out=thr_sb[:], in_=thr_d[:])
            aff_sb = singles.tile([COUT, 2], F32)
            nc.sync.dma_start(out=aff_sb[:], in_=aff_d.rearrange("s co -> co s"))

            c_sb = singles.tile([128, 512], F32)    # running cumsum of pooled
            nc.vector.memset(c_sb[:], 0.0)
            g_sb = [singles.tile([128, 512], BF16, name=f"g{i}") for i in range(2)]
            nc.vector.memset(g_sb[0][:], 0.0)

            # --- spiking path: 20 timesteps ---
            for t in range(T):
                x3 = x3p.tile([96, HW], MM_DT, tag="x3")
                # split load across 2 DMAs for queue parallelism
                nc.sync.dma_start(out=x3[:, :HW // 2], in_=x3_d[t, :, :HW // 2])
                nc.sync.dma_start(out=x3[:, HW // 2:], in_=x3_d[t, :, HW // 2:])

                ph = php.tile([COUT, 2048], F32, tag="ph")
                _conv_image(nc, tc, pools, wt_sb[:, 0], x3, ph)

                pv = pvp.tile([COUT, 1024], F32, tag="pv")
                _pool_v(nc, ph, pv)

                # pack [64,1024] -> [128,512]
                pk = pkp.tile([128, 512], F32, tag="pk")
                nc.sync.dma_start(out=pk[:COUT, :], in_=pv[:, :512])
                nc.sync.dma_start(out=pk[COUT:, :], in_=pv[:, 512:])

                # scan step
                nc.vector.tensor_add(c_sb[:], c_sb[:], pk[:])
                f = fp_.tile([128, 512], BF16, tag="f")
                nc.vector.tensor_scalar(
                    f[:], c_sb[:], thr_sb[:, t:t + 1], None, mybir.AluOpType.is_ge,
                )
                go, gn = g_sb[t % 2], g_sb[(t + 1) % 2]
                nc.vector.tensor_tensor(gn[:], go[:], f[:], mybir.AluOpType.max)
                s = spp.tile([128, 512], BF16, tag="s")
                nc.vector.tensor_tensor(s[:], gn[:], go[:], mybir.AluOpType.subtract)
                nc.sync.dma_start(out=spike_d[t], in_=s[:])

            # count = g after step T-1
            nc.sync.dma_start(out=count_d[:], in_=g_sb[T % 2][:])

            # --- ANN path ---
            x3a = x3p.tile([96, HW], MM_DT, tag="x3")
            nc.sync.dma_start(out=x3a[:, :HW // 2], in_=xsc3_d[:, :HW // 2])
            nc.sync.dma_start(out=x3a[:, HW // 2:], in_=xsc3_d[:, HW // 2:])

            ya = singles.tile([COUT, HW], F32)
            for g in range(N_GROUP):
                ps = psum.tile([COUT, GROUP * CHUNK], F32)
                for ci in range(GROUP):
                    cidx = g * GROUP + ci
                    base = cidx * CHUNK
                    for j, ky in enumerate((1, 0, 2)):
                        off = base + (ky - 1) * W
                        lo = max(off, 0)
                        hi = min(off + CHUNK, HW)
                        n = hi - lo
                        po = ci * CHUNK + (lo - off)
                        nc.tensor.matmul(
                            ps[:, po:po + n],
                            lhsT=_mm(wt_sb[:, 1, ky, :]),
                            rhs=_mm(x3a[:, lo:hi]),
                            start=(j == 0),
                            stop=(j == 2),
                        )
                # BN affine during PSUM evacuation (before pool: gamma may be <0)
                nc.scalar.activation(
                    ya[:, g * GROUP * CHUNK:(g + 1) * GROUP * CHUNK], ps[:],
                    mybir.ActivationFunctionType.Identity,
                    bias=aff_sb[:, 1:2], scale=aff_sb[:, 0:1],
                )
            pha = php.tile([COUT, 2048], F32, tag="ph")
            yav = ya.rearrange("p (f two) -> p f two", two=2)
            nc.vector.tensor_tensor(pha[:], yav[:, :, 0], yav[:, :, 1],
                                    mybir.AluOpType.max)
            pva = pvp.tile([COUT, 1024], F32, tag="pv")
            _pool_v(nc, pha, pva)
            anno = singles.tile([COUT, 1024], F32)
            nc.scalar.activation(anno[:], pva[:], mybir.ActivationFunctionType.Relu)
            nc.sync.dma_start(out=ann_d[:], in_=anno[:])

    nc.compile()
    return nc


def _prep_core(x_st_n, xsc_n, wN, w, thr, aff):
    """Build per-core input map. x_st_n: (T,CIN,H,W), xsc_n: (CIN,H,W)."""
    def shift3(x):  # x: (..., CIN, H, W) -> (..., 96, H*W) kx-shifted copies
        lead = x.shape[:-3]
        out = np.zeros(lead + (96, H, W), np.float32)
        out[..., 0:32, :, 1:] = x[..., :, :, :-1]   # kx=0 reads w-1
        out[..., 32:64, :, :] = x
        out[..., 64:96, :, :-1] = x[..., :, :, 1:]  # kx=2 reads w+1
        return out.reshape(lead + (96, HW))

    def wt_layout(wmat):  # (COUT,CIN,3,3) -> [96,3,COUT] = [(kx cin), ky, co]
        return np.ascontiguousarray(wmat.transpose(3, 1, 2, 0).reshape(96, 3, COUT))

    return {
        "x3": shift3(x_st_n),
        "xsc3": shift3(xsc_n),
        "wt": np.stack([wt_layout(wN), wt_layout(w)]).astype(np.float32),
        "thr": thr,
        "aff": aff,
    }


def kernel(input_feature_st, input_features_sc, conv_w, conv_b, gamma, beta,
           running_mean, running_var):
    global _COMPILED
    x_st = np.asarray(input_feature_st, np.float32)
    x_sc = np.asarray(input_features_sc, np.float32)
    w = np.asarray(conv_w, np.float32)
    b = np.asarray(conv_b, np.float32)
    gamma = np.asarray(gamma, np.float32)
    beta = np.asarray(beta, np.float32)
    rm = np.asarray(running_mean, np.float32)
    rv = np.asarray(running_var, np.float32)

    # BN folding (host, fp32 — matches reference math)
    ratio = gamma / np.sqrt(rv)                    # spiking path (no eps)
    wN = w * ratio[:, None, None, None]
    bN = (b - rm) * ratio + beta
    scale_ann = gamma / np.sqrt(rv + EPS)
    bias_ann = (b - rm) * scale_ann + beta

    # thr[p, t] = 1 - (t+1)*bN[p%64]
    tt = np.arange(1, T + 1, dtype=np.float32)
    thr64 = np.float32(1.0) - bN[:, None] * tt[None, :]
    thr = np.concatenate([thr64, thr64], axis=0).astype(np.float32)  # (128, T)
    aff = np.stack([scale_ann, bias_ann]).astype(np.float32)         # (2, 64)

    if _COMPILED is None:
        _COMPILED = build_nc()
    nc = _COMPILED

    in_maps = [_prep_core(x_st[n], x_sc[n], wN, w, thr, aff) for n in range(N)]
    res = run_bass_kernel_spmd(nc, in_maps, core_ids=list(range(N)))

    spike_out = np.empty((N, T, COUT, OH, OW), np.float32)
    spike_count = np.empty((N, COUT, OH, OW), np.float32)
    ann_out = np.empty((N, COUT, OH, OW), np.float32)
    for n in range(N):
        r = res.results[n]
        # unpack [128,512]: p = two*64+co, f = hh*32+u, v = two*16+hh
        sp = r["spike"].astype(np.float32).reshape(T, 2, COUT, 16, OW)
        spike_out[n] = sp.transpose(0, 2, 1, 3, 4).reshape(T, COUT, OH, OW)
        cn = r["count"].astype(np.float32).reshape(2, COUT, 16, OW)
        spike_count[n] = cn.transpose(1, 0, 2, 3).reshape(COUT, OH, OW)
        ann_out[n] = r["ann"].reshape(COUT, OH, OW)

    return spike_out, spike_count, ann_out
